# revision 33
# baseline (speedup 1.0000x reference)
"""Trainium2 Bass kernel for nn_BiasedConLoss (supervised-contrastive biased loss).

Math (validated against the jax reference to ~2e-6 rel):
  X = concat(features, features_cr)            [M=8192, D=256], rows L2-normalized
  A = X @ X.T  (raw dots), sims = A / T
  row max = diag(A)/T (diag==1 dominates off-diag cos sims)
  The only O(M^2) quantity needed is Q_i = sum_j exp((A_ij - 1)/T).
  Everything else (pos_dot via matvec, partner dots, diag, P/U) is O(M*D)
  and computed on host in float64.

Device (8 NeuronCores, SPMD):
  Each core owns 1024 rows of A. It gets xin [256, 9216] =
  per k-half (128 d's): [ XT_k cols 0:512 | XT_k cols g0:g0+1024 (own rows) |
                          XT_k cols 512:8192 ]
  The "bigA" tile (first 640 cols) holds both the first rhs col-tile and the
  r0 lhsT columns, so the first matmul of each k-group waits on ONE DMA
  semaphore (walrus allows only one sync-wait per instruction here).
  The GEMM runs in fp16 (inputs pre-rounded on host, products exact in the
  fp32 PSUM accumulator, 1 cyc/row on the PE vs ~1.85 for f32r).
  Per col chunk c (2048 cols, outer) and row tile r (128 rows): 8 fp16
  matmuls into a [128,2048] PSUM tile (2 k-chunks x 4 col tiles), then one
  ScalarE activation Exp(in*1/T - 1/T) in-place with accum_out giving the
  row-sum partials. Output stats [128, 33]: partials per (chunk, row tile).
"""
import numpy as np

import concourse.bass as bass
import concourse.tile as tile
from concourse import mybir
from concourse.bass_utils import run_bass_kernel_spmd
from concourse.vector_clock import ScopedClock, VectorClock

F32 = mybir.dt.float32
F16 = mybir.dt.float16

T = 0.07
N = 4096
D = 256
M = 2 * N           # 8192
NCORES = 8
ROWS_PER_CORE = M // NCORES          # 1024
NR = ROWS_PER_CORE // 128            # 8 row tiles per core
NJ = M // 512                        # 16 col tiles of 512
NT = 4                               # col tiles per chunk
NCHUNK = NJ // NT                    # 4 chunks of 2048 cols
XIN_COLS = 512 + ROWS_PER_CORE + (M - 512)   # 9216


_SELF_SEM_PREFIX = {
    mybir.EngineType.PE: "PE_",
    mybir.EngineType.Activation: "Activation_",
    mybir.EngineType.DVE: "DVE_",
}


class _SplitDrainTileContext(tile.TileContext):
    """Two walrus-compat adjustments for this toolchain (which allows only
    ONE sync-wait per instruction):

    1. Strip same-engine semaphore self-waits from PE/ACT/DVE instructions.
       These engines execute and complete their instruction streams strictly
       in order (PE matmuls are pc-monotone; ACT/DVE are strict-FIFO queues),
       so a wait on the engine's own completion semaphore is redundant with
       program order. Tile emits them conservatively for WAW/WAR hazards
       across PSUM-slot reuse.
    2. Split the kernel-tail drain's sem waits across many Drain
       instructions."""

    def _lower_ordered_insts(self, postordered_blocks):
        for insts in postordered_blocks.values():
            for inst in insts:
                si = getattr(inst, "sync_info", None)
                if si is None or not si.on_wait:
                    continue
                prefix = _SELF_SEM_PREFIX.get(inst.engine)
                kept = si.on_wait
                if prefix is not None:
                    kept = [
                        w for w in kept
                        if not (w.ant_name or "").startswith(prefix)
                    ]
                if (
                    inst.engine == mybir.EngineType.Pool
                    and type(inst).__name__ == "InstDMACopy"
                ):
                    # The only Pool DMA here is the stats store, whose sole
                    # data dep is the DVE-written stats tile; DMASW waits on
                    # it are same-queue FIFO ordering (redundant in-order).
                    kept = [
                        w for w in kept
                        if not (w.ant_name or "").startswith("DMASW")
                    ]
                if len(kept) != len(si.on_wait):
                    si.on_wait = kept
        return super()._lower_ordered_insts(postordered_blocks)

    def _drain_and_barrier(self, tick_clock, wait_clock):
        full = tick_clock.global_clock
        n = len(full)
        procs = [p for p in range(n) if full[p] > 0]
        for p in procs:
            vec = [full[q] if q == p else 0 for q in range(n)]
            d = self.nc.sync.drain()
            wait_clock.add_sem_waits(d.ins, ScopedClock({None: VectorClock(vec)}))
        if not procs:
            d = self.nc.sync.drain()
            wait_clock.add_sem_waits(
                d.ins, ScopedClock({None: tick_clock.global_clock})
            )
        self.nc.all_engine_barrier()
        assert self.sems is not None
        popped = self.nc._tile_sem_poison_stack.pop()
        assert popped is self._sem_poison
        self.nc.clear_and_free_semaphores(list(self.sems.allocated().values()))
        self.nc.all_engine_barrier()


def _build():
    nc = bass.Bass("TRN2", target_bir_lowering=False, debug=False,
                   num_swdge_queues=1)
    xin = nc.dram_tensor("xin", [2 * 128, XIN_COLS], F16, kind="ExternalInput").ap()
    stats = nc.dram_tensor(
        "stats", [128, NCHUNK * NR], F32, kind="ExternalOutput"
    ).ap()

    bias_t = nc.alloc_sbuf_tensor("bias_const", [128, 1], F32)
    warm_t = nc.alloc_sbuf_tensor("warm_zeros", [128, 512], F16)

    with _SplitDrainTileContext(nc) as tc:
        ones = nc.const_aps.tensor(1.0, (128, 1), mybir.dt.float32)
        nc.scalar.mul(bias_t.ap(), ones, -1.0 / T)
        nc.vector.memset(warm_t.ap(), 0.0)
        with tc.tile_pool(name="big", bufs=2) as big_pool, \
             tc.tile_pool(name="rhs", bufs=2 * (NJ - 1)) as rhs_pool, \
             tc.tile_pool(name="acc", bufs=NCHUNK * NR) as acc_pool, \
             tc.tile_pool(name="stat", bufs=1) as stat_pool, \
             tc.tile_pool(name="escr", bufs=2) as escr_pool, \
             tc.tile_pool(name="ps", bufs=2, space="PSUM") as ps_pool:

            # bigA per k-half: [rhs j0 (512) | lhsT r0 (128)]; bigB: lhsT r1..r7.
            # Packing lhsT with rhs j0 keeps the first matmul of each k-group
            # down to ONE DMA-sem wait; the split keeps the gating transfer
            # small so PE starts early.
            bigA, bigB = [], []
            rhs = {}

            # Per-tile DMAs, all on the Sync engine's HWDGE ring, issued in
            # consumption order. Each dispatch costs ~0.6us on SP, but the
            # fine granularity supplies tiles incrementally so PE starts
            # ~11us in and never starves (measured best vs consolidated
            # transfers, whose ring serialization delays chunk 0).
            R = {}

            def dma_rhs(j, k):
                rt = rhs_pool.tile([128, 512], F16, tag="rhs")
                c0 = 512 + ROWS_PER_CORE + 512 * (j - 1)
                nc.sync.dma_start(
                    out=rt[:], in_=xin[128 * k:128 * (k + 1), c0:c0 + 512]
                )
                R[(j, k)] = rt

            for k in range(2):
                bt = big_pool.tile([128, 640], F16, tag="bigA")
                nc.sync.dma_start(out=bt[:], in_=xin[128 * k:128 * (k + 1), 0:640])
                bigA.append(bt)
            for k in range(2):
                dma_rhs(1, k)
            for k in range(2):
                bt = big_pool.tile([128, ROWS_PER_CORE - 128], F16, tag="bigB")
                nc.sync.dma_start(
                    out=bt[:],
                    in_=xin[128 * k:128 * (k + 1), 640:512 + ROWS_PER_CORE],
                )
                bigB.append(bt)
            for j in range(2, NJ):
                for k in range(2):
                    dma_rhs(j, k)

            def rhs_ap(k, j):
                if j == 0:
                    return bigA[k][:, 0:512]
                return R[(j, k)][:]

            def lhsT_ap(k, r):
                if r == 0:
                    return bigA[k][:, 512:640]
                return bigB[k][:, 128 * (r - 1):128 * r]

            stat_sb = stat_pool.tile([128, NCHUNK * NR], F32)

            # PE warm-up: ~3.5us of zero matmuls on a preamble-initialized
            # const tile. No input deps -> starts immediately, releases the
            # HAM clock throttle before the first real matmul arrives.
            warm_ap = warm_t.ap()
            ps_warm = ps_pool.tile([128, 512 * NT], F32, tag="ps")
            for _ in range(4):
                nc.tensor.matmul(
                    ps_warm[0:1, 0:512],
                    lhsT=warm_ap[:, 0:1],
                    rhs=warm_ap[:],
                    start=True, stop=True,
                    skip_group_check=True,
                )

            def do_act(ps_ap, col):
                # exp output itself is dead (only the accumulated row-sum
                # matters) -> write it to SBUF scratch: ScalarE PSUM-source /
                # SBUF-dest has lower per-op overhead than in-place PSUM.
                acc = acc_pool.tile([128, 1], F32)
                scr = escr_pool.tile([128, 512 * NT], F16, tag="escr")
                nc.scalar.activation(
                    out=scr[:], in_=ps_ap,
                    func=mybir.ActivationFunctionType.Exp,
                    bias=bias_t.ap(), scale=1.0 / T,
                    accum_out=acc[:],
                )
                nc.vector.tensor_copy(stat_sb[:, col:col + 1], acc[:])

            # Column-chunk-outer: chunk c only needs its 8 input tiles, so
            # chunk 0's compute (~18us) hides the remaining input DMA.
            for c in range(NCHUNK):
                for r in range(NR):
                    ps = ps_pool.tile([128, 512 * NT], F32)
                    # WAR-absorber: the first MM of a new chunk would need
                    # BOTH the PSUM-reuse WAR sem and a fresh DMA sem ->
                    # 2 waits (walrus allows 1). Absorb the WAR wait with one
                    # matmul on already-observed tiles; the real k0 group
                    # overwrites it (start=True).
                    if c > 0 and r == 0:
                        nc.tensor.matmul(
                            ps[:, 0:512],
                            lhsT=lhsT_ap(0, 0),
                            rhs=rhs_ap(0, 0),
                            start=True, stop=True,
                            skip_group_check=True,
                        )
                    for k in range(2):
                        lhsT = lhsT_ap(k, r)
                        for t in range(NT):
                            j = NT * c + t
                            nc.tensor.matmul(
                                ps[:, 512 * t:512 * (t + 1)],
                                lhsT=lhsT,
                                rhs=rhs_ap(k, j),
                                start=(k == 0), stop=(k == 1),
                                skip_group_check=(c > 0 and r == 0),
                            )
                    do_act(ps[:], c * NR + r)
            # SWDGE: keeps the output DMAs off the busy HWDGE queues so they
            # carry only the DVE wait (1-wait limit). Split so the bulk store
            # overlaps the last chunk's compute.
            nc.gpsimd.dma_start(out=stats[:, 0:24], in_=stat_sb[:, 0:24])
            nc.gpsimd.dma_start(
                out=stats[:, 24:NCHUNK * NR],
                in_=stat_sb[:, 24:NCHUNK * NR],
            )
    return nc


_NC_CACHE = None


def _get_nc():
    global _NC_CACHE
    if _NC_CACHE is None:
        _NC_CACHE = _build()
    return _NC_CACHE


def kernel(labels, all_features, all_features_cr, _trace=False):
    labels = np.asarray(labels)
    f = np.asarray(all_features, dtype=np.float32)
    f_cr = np.asarray(all_features_cr, dtype=np.float32)

    # fp16 GEMM on device: products of fp16 values are exact in the fp32
    # PSUM accumulator, so host float64 math over the SAME fp16-rounded
    # values matches the device GEMM to fp32-accumulation noise.
    X16 = np.concatenate([f, f_cr], axis=0).astype(np.float16)   # [M, D]
    X32 = X16.astype(np.float32)
    XT = np.ascontiguousarray(X16.T)                       # [D, M] fp16

    in_maps = []
    for c in range(NCORES):
        g0 = c * ROWS_PER_CORE
        xin = np.empty((2 * 128, XIN_COLS), dtype=np.float16)
        for k in range(2):
            rows = slice(128 * k, 128 * (k + 1))
            xin[rows, 0:512] = XT[rows, 0:512]
            xin[rows, 512:512 + ROWS_PER_CORE] = XT[rows, g0:g0 + ROWS_PER_CORE]
            xin[rows, 512 + ROWS_PER_CORE:] = XT[rows, 512:M]
        in_maps.append({"xin": xin})

    nc = _get_nc()
    res = run_bass_kernel_spmd(
        nc, in_maps, core_ids=list(range(NCORES)), trace=_trace
    )
    kernel.last_exec_time_ns = res.exec_time_ns
    kernel.last_trace = res.instructions_and_trace

    # stats[p, c*NR + r] = sum_{j in chunk c} exp((A[g0+128r+p, j] - 1)/T)
    Q = np.empty(M, dtype=np.float64)
    for core in range(NCORES):
        st = res.results[core]["stats"].astype(np.float64)   # [128, 32]
        per_row = st.reshape(128, NCHUNK, NR).sum(axis=1)
        for r in range(NR):
            i0 = core * ROWS_PER_CORE + r * 128
            Q[i0:i0 + 128] = per_row[:, r]

    # ---- host epilogue (float64, O(M*D)) ----
    X = X32.astype(np.float64)
    lab = np.asarray(labels)
    all_labels = np.concatenate([lab, lab]).astype(np.float64)
    pos_f = (all_labels == 1).astype(np.float64)
    neg_f = 1.0 - pos_f
    P = pos_f.sum()
    U = neg_f.sum()

    d = np.sum(X * X, axis=1)                 # diag of A
    row_sum = Q * np.exp((1.0 - d) / T)       # = 1 + sum_{j!=i} exp((A_ij-d_i)/T)
    row_logsum = np.log(row_sum)

    w_pos = pos_f @ X
    pos_dot_raw = X @ w_pos
    spos = (pos_dot_raw - P * d) / T
    sup_row = spos - M * row_logsum
    loss_sup = np.sum(pos_f * (-sup_row / P)) / P

    partner = np.sum(X * np.roll(X, -N, axis=0), axis=1)
    unsup_row = (partner - d) / T - M * row_logsum
    loss_unsup = np.sum(neg_f * (-unsup_row / U)) / U

    return (np.float32(loss_sup), np.float32(loss_unsup))


# revision 34
# speedup vs baseline: 1.0227x; 1.0227x over previous
"""Trainium2 Bass kernel for nn_BiasedConLoss (supervised-contrastive biased loss).

Math (validated against the jax reference to ~2e-6 rel):
  X = concat(features, features_cr)            [M=8192, D=256], rows L2-normalized
  A = X @ X.T  (raw dots), sims = A / T
  row max = diag(A)/T (diag==1 dominates off-diag cos sims)
  The only O(M^2) quantity needed is Q_i = sum_j exp((A_ij - 1)/T).
  Everything else (pos_dot via matvec, partner dots, diag, P/U) is O(M*D)
  and computed on host in float64.

Device (8 NeuronCores, SPMD):
  Each core owns 1024 rows of A. It gets xin [256, 9216] =
  per k-half (128 d's): [ XT_k cols 0:512 | XT_k cols g0:g0+1024 (own rows) |
                          XT_k cols 512:8192 ]
  The "bigA" tile (first 640 cols) holds both the first rhs col-tile and the
  r0 lhsT columns, so the first matmul of each k-group waits on ONE DMA
  semaphore (walrus allows only one sync-wait per instruction here).
  The GEMM runs in fp16 (inputs pre-rounded on host, products exact in the
  fp32 PSUM accumulator, 1 cyc/row on the PE vs ~1.85 for f32r).
  Per col chunk c (2048 cols, outer) and row tile r (128 rows): 8 fp16
  matmuls into a [128,2048] PSUM tile (2 k-chunks x 4 col tiles), then one
  ScalarE activation Exp(in*1/T - 1/T) in-place with accum_out giving the
  row-sum partials. Output stats [128, 33]: partials per (chunk, row tile).
"""
import numpy as np

import concourse.bass as bass
import concourse.tile as tile
from concourse import mybir
from concourse.bass_utils import run_bass_kernel_spmd
from concourse.vector_clock import ScopedClock, VectorClock

F32 = mybir.dt.float32
F16 = mybir.dt.float16

T = 0.07
N = 4096
D = 256
M = 2 * N           # 8192
NCORES = 8
ROWS_PER_CORE = M // NCORES          # 1024
NR = ROWS_PER_CORE // 128            # 8 row tiles per core
NJ = M // 512                        # 16 col tiles of 512
NT = 4                               # col tiles per chunk
NCHUNK = NJ // NT                    # 4 chunks of 2048 cols
XIN_COLS = 512 + ROWS_PER_CORE + (M - 512)   # 9216


_SELF_SEM_PREFIX = {
    mybir.EngineType.PE: "PE_",
    mybir.EngineType.Activation: "Activation_",
    mybir.EngineType.DVE: "DVE_",
}


class _SplitDrainTileContext(tile.TileContext):
    """Two walrus-compat adjustments for this toolchain (which allows only
    ONE sync-wait per instruction):

    1. Strip same-engine semaphore self-waits from PE/ACT/DVE instructions.
       These engines execute and complete their instruction streams strictly
       in order (PE matmuls are pc-monotone; ACT/DVE are strict-FIFO queues),
       so a wait on the engine's own completion semaphore is redundant with
       program order. Tile emits them conservatively for WAW/WAR hazards
       across PSUM-slot reuse.
    2. Split the kernel-tail drain's sem waits across many Drain
       instructions."""

    def _lower_ordered_insts(self, postordered_blocks):
        for insts in postordered_blocks.values():
            for inst in insts:
                si = getattr(inst, "sync_info", None)
                if si is None or not si.on_wait:
                    continue
                prefix = _SELF_SEM_PREFIX.get(inst.engine)
                kept = si.on_wait
                if prefix is not None:
                    kept = [
                        w for w in kept
                        if not (w.ant_name or "").startswith(prefix)
                    ]
                if (
                    inst.engine == mybir.EngineType.Pool
                    and type(inst).__name__ == "InstDMACopy"
                ):
                    # The only Pool DMA here is the stats store, whose sole
                    # data dep is the DVE-written stats tile; DMASW waits on
                    # it are same-queue FIFO ordering (redundant in-order).
                    kept = [
                        w for w in kept
                        if not (w.ant_name or "").startswith("DMASW")
                    ]
                if len(kept) != len(si.on_wait):
                    si.on_wait = kept
        return super()._lower_ordered_insts(postordered_blocks)

    def _drain_and_barrier(self, tick_clock, wait_clock):
        full = tick_clock.global_clock
        n = len(full)
        procs = [p for p in range(n) if full[p] > 0]
        for p in procs:
            vec = [full[q] if q == p else 0 for q in range(n)]
            d = self.nc.sync.drain()
            wait_clock.add_sem_waits(d.ins, ScopedClock({None: VectorClock(vec)}))
        if not procs:
            d = self.nc.sync.drain()
            wait_clock.add_sem_waits(
                d.ins, ScopedClock({None: tick_clock.global_clock})
            )
        self.nc.all_engine_barrier()
        assert self.sems is not None
        popped = self.nc._tile_sem_poison_stack.pop()
        assert popped is self._sem_poison
        self.nc.clear_and_free_semaphores(list(self.sems.allocated().values()))
        self.nc.all_engine_barrier()


def _build():
    nc = bass.Bass("TRN2", target_bir_lowering=False, debug=False,
                   num_swdge_queues=1)
    xin = nc.dram_tensor("xin", [2 * 128, XIN_COLS], F16, kind="ExternalInput").ap()
    stats = nc.dram_tensor(
        "stats", [128, NCHUNK * NR], F32, kind="ExternalOutput"
    ).ap()

    bias_t = nc.alloc_sbuf_tensor("bias_const", [128, 1], F32)
    warm_t = nc.alloc_sbuf_tensor("warm_zeros", [128, 512], F16)

    with _SplitDrainTileContext(nc) as tc:
        ones = nc.const_aps.tensor(1.0, (128, 1), mybir.dt.float32)
        nc.scalar.mul(bias_t.ap(), ones, -1.0 / T)
        nc.vector.memset(warm_t.ap(), 0.0)
        with tc.tile_pool(name="big", bufs=2) as big_pool, \
             tc.tile_pool(name="rhs", bufs=2 * (NJ - 1)) as rhs_pool, \
             tc.tile_pool(name="acc", bufs=NCHUNK * NR) as acc_pool, \
             tc.tile_pool(name="stat", bufs=1) as stat_pool, \
             tc.tile_pool(name="ps", bufs=2, space="PSUM") as ps_pool:

            # bigA per k-half: [rhs j0 (512) | lhsT r0 (128)]; bigB: lhsT r1..r7.
            # Packing lhsT with rhs j0 keeps the first matmul of each k-group
            # down to ONE DMA-sem wait; the split keeps the gating transfer
            # small so PE starts early.
            bigA, bigB = [], []
            rhs = {}

            # Per-tile DMAs, all on the Sync engine's HWDGE ring, issued in
            # consumption order. Each dispatch costs ~0.6us on SP, but the
            # fine granularity supplies tiles incrementally so PE starts
            # ~11us in and never starves (measured best vs consolidated
            # transfers, whose ring serialization delays chunk 0).
            R = {}

            def dma_rhs(j, k):
                rt = rhs_pool.tile([128, 512], F16, tag="rhs")
                c0 = 512 + ROWS_PER_CORE + 512 * (j - 1)
                nc.sync.dma_start(
                    out=rt[:], in_=xin[128 * k:128 * (k + 1), c0:c0 + 512]
                )
                R[(j, k)] = rt

            for k in range(2):
                bt = big_pool.tile([128, 640], F16, tag="bigA")
                nc.sync.dma_start(out=bt[:], in_=xin[128 * k:128 * (k + 1), 0:640])
                bigA.append(bt)
            for k in range(2):
                dma_rhs(1, k)
            for k in range(2):
                bt = big_pool.tile([128, ROWS_PER_CORE - 128], F16, tag="bigB")
                nc.sync.dma_start(
                    out=bt[:],
                    in_=xin[128 * k:128 * (k + 1), 640:512 + ROWS_PER_CORE],
                )
                bigB.append(bt)
            for j in range(2, NJ):
                for k in range(2):
                    dma_rhs(j, k)

            def rhs_ap(k, j):
                if j == 0:
                    return bigA[k][:, 0:512]
                return R[(j, k)][:]

            def lhsT_ap(k, r):
                if r == 0:
                    return bigA[k][:, 512:640]
                return bigB[k][:, 128 * (r - 1):128 * r]

            stat_sb = stat_pool.tile([128, NCHUNK * NR], F32)

            # PE warm-up: ~3.5us of zero matmuls on a preamble-initialized
            # const tile. No input deps -> starts immediately, releases the
            # HAM clock throttle before the first real matmul arrives.
            warm_ap = warm_t.ap()
            ps_warm = ps_pool.tile([128, 512 * NT], F32, tag="ps")
            for _ in range(4):
                nc.tensor.matmul(
                    ps_warm[0:1, 0:512],
                    lhsT=warm_ap[:, 0:1],
                    rhs=warm_ap[:],
                    start=True, stop=True,
                    skip_group_check=True,
                )

            def do_act(ps_ap, col):
                # exp output is dead (only the accumulated row-sum matters);
                # in-place PSUM write is the cheapest legal destination.
                acc = acc_pool.tile([128, 1], F32)
                nc.scalar.activation(
                    out=ps_ap, in_=ps_ap,
                    func=mybir.ActivationFunctionType.Exp,
                    bias=bias_t.ap(), scale=1.0 / T,
                    accum_out=acc[:],
                )
                nc.vector.tensor_copy(stat_sb[:, col:col + 1], acc[:])

            # Column-chunk-outer: chunk c only needs its 8 input tiles, so
            # chunk 0's compute (~18us) hides the remaining input DMA.
            for c in range(NCHUNK):
                for r in range(NR):
                    ps = ps_pool.tile([128, 512 * NT], F32)
                    # WAR-absorber: the first MM of a new chunk would need
                    # BOTH the PSUM-reuse WAR sem and a fresh DMA sem ->
                    # 2 waits (walrus allows 1). Absorb the WAR wait with one
                    # matmul on already-observed tiles; the real k0 group
                    # overwrites it (start=True).
                    if c > 0 and r == 0:
                        nc.tensor.matmul(
                            ps[:, 0:512],
                            lhsT=lhsT_ap(0, 0),
                            rhs=rhs_ap(0, 0),
                            start=True, stop=True,
                            skip_group_check=True,
                        )
                    for k in range(2):
                        lhsT = lhsT_ap(k, r)
                        for t in range(NT):
                            j = NT * c + t
                            nc.tensor.matmul(
                                ps[:, 512 * t:512 * (t + 1)],
                                lhsT=lhsT,
                                rhs=rhs_ap(k, j),
                                start=(k == 0), stop=(k == 1),
                                skip_group_check=(c > 0 and r == 0),
                            )
                    do_act(ps[:], c * NR + r)
            # SWDGE: keeps the output DMAs off the busy HWDGE queues so they
            # carry only the DVE wait (1-wait limit). Split so the bulk store
            # overlaps the last chunk's compute.
            nc.gpsimd.dma_start(out=stats[:, 0:24], in_=stat_sb[:, 0:24])
            nc.gpsimd.dma_start(
                out=stats[:, 24:NCHUNK * NR],
                in_=stat_sb[:, 24:NCHUNK * NR],
            )
    return nc


_NC_CACHE = None


def _get_nc():
    global _NC_CACHE
    if _NC_CACHE is None:
        _NC_CACHE = _build()
    return _NC_CACHE


def kernel(labels, all_features, all_features_cr, _trace=False):
    labels = np.asarray(labels)
    f = np.asarray(all_features, dtype=np.float32)
    f_cr = np.asarray(all_features_cr, dtype=np.float32)

    # fp16 GEMM on device: products of fp16 values are exact in the fp32
    # PSUM accumulator, so host float64 math over the SAME fp16-rounded
    # values matches the device GEMM to fp32-accumulation noise.
    X16 = np.concatenate([f, f_cr], axis=0).astype(np.float16)   # [M, D]
    X32 = X16.astype(np.float32)
    XT = np.ascontiguousarray(X16.T)                       # [D, M] fp16

    in_maps = []
    for c in range(NCORES):
        g0 = c * ROWS_PER_CORE
        xin = np.empty((2 * 128, XIN_COLS), dtype=np.float16)
        for k in range(2):
            rows = slice(128 * k, 128 * (k + 1))
            xin[rows, 0:512] = XT[rows, 0:512]
            xin[rows, 512:512 + ROWS_PER_CORE] = XT[rows, g0:g0 + ROWS_PER_CORE]
            xin[rows, 512 + ROWS_PER_CORE:] = XT[rows, 512:M]
        in_maps.append({"xin": xin})

    nc = _get_nc()
    res = run_bass_kernel_spmd(
        nc, in_maps, core_ids=list(range(NCORES)), trace=_trace
    )
    kernel.last_exec_time_ns = res.exec_time_ns
    kernel.last_trace = res.instructions_and_trace

    # stats[p, c*NR + r] = sum_{j in chunk c} exp((A[g0+128r+p, j] - 1)/T)
    Q = np.empty(M, dtype=np.float64)
    for core in range(NCORES):
        st = res.results[core]["stats"].astype(np.float64)   # [128, 32]
        per_row = st.reshape(128, NCHUNK, NR).sum(axis=1)
        for r in range(NR):
            i0 = core * ROWS_PER_CORE + r * 128
            Q[i0:i0 + 128] = per_row[:, r]

    # ---- host epilogue (float64, O(M*D)) ----
    X = X32.astype(np.float64)
    lab = np.asarray(labels)
    all_labels = np.concatenate([lab, lab]).astype(np.float64)
    pos_f = (all_labels == 1).astype(np.float64)
    neg_f = 1.0 - pos_f
    P = pos_f.sum()
    U = neg_f.sum()

    d = np.sum(X * X, axis=1)                 # diag of A
    row_sum = Q * np.exp((1.0 - d) / T)       # = 1 + sum_{j!=i} exp((A_ij-d_i)/T)
    row_logsum = np.log(row_sum)

    w_pos = pos_f @ X
    pos_dot_raw = X @ w_pos
    spos = (pos_dot_raw - P * d) / T
    sup_row = spos - M * row_logsum
    loss_sup = np.sum(pos_f * (-sup_row / P)) / P

    partner = np.sum(X * np.roll(X, -N, axis=0), axis=1)
    unsup_row = (partner - d) / T - M * row_logsum
    loss_unsup = np.sum(neg_f * (-unsup_row / U)) / U

    return (np.float32(loss_sup), np.float32(loss_unsup))
